# revision 11
# baseline (speedup 1.0000x reference)
"""BinaryConv2d (3x3, SAME, NHWC) on 8 trn2 NeuronCores.

Sharding: data-parallel over batch — 2 images per core; the tiny binarized
weight tensor is replicated. Per core, the two images are packed on the two
64-partition halves of SBUF so each 3x3-tap matmul pair (K=64 contraction =
C_in) runs concurrently on disjoint row-groups of the 128x128 PE array.

Layout trick: the conv is evaluated on a flat q-grid over a zero-padded
226-wide plane, so each of the 9 taps is a pure free-dim offset
(dh*226 + dw) into the same SBUF x tile; row-crossing outputs land in 2
garbage columns per row that the host discards.
"""

import sys

for _p in ("/opt/trn_rl_repo",):
    if _p not in sys.path:
        sys.path.insert(0, _p)

import ml_dtypes
import numpy as np

BF16 = ml_dtypes.bfloat16

N_CORES = 8
IMG_PER_CORE = 2
H = W_IMG = 224
C_IN, C_OUT = 64, 128
PR, PC = 227, 226  # padded plane: 226 rows of data + 1 extra zero row
PLANE = PR * PC  # 51302
QOUT = H * PC  # 50624 q-positions per image (2 garbage cols per row)
NTAPS = 9
SLOT = 512  # q-positions per matmul (one PSUM bank of fp32)
N_SLOTS = (QOUT + SLOT - 1) // SLOT  # 99 (last slot = 448)
SLOTS_PER_CHUNK = 16
FIRST_CHUNK_SLOTS = 2  # small first chunk so the PE starts early
HALO = 2 * PC + 2  # 454: max tap offset
CHUNK_Q = SLOTS_PER_CHUNK * SLOT
XTILE_COLS = CHUNK_Q + HALO
STAGE_SLOTS = 8
STAGE_Q = STAGE_SLOTS * SLOT


def _chunk_starts():
    starts = [0, FIRST_CHUNK_SLOTS]
    while starts[-1] + SLOTS_PER_CHUNK < N_SLOTS:
        starts.append(starts[-1] + SLOTS_PER_CHUNK)
    return starts

_COMPILED = None
_LAST_RES = None


def _build():
    import concourse.mybir as mybir
    import concourse.tile as tile
    from concourse import bacc

    nc = bacc.Bacc(
        "TRN2", target_bir_lowering=False, debug=False, num_devices=N_CORES
    )
    x_d = nc.dram_tensor("x", [128, PLANE], mybir.dt.bfloat16, kind="ExternalInput")
    w_d = nc.dram_tensor(
        "w", [128, NTAPS * 128], mybir.dt.bfloat16, kind="ExternalInput"
    )
    b_d = nc.dram_tensor("b", [128, 1], mybir.dt.float32, kind="ExternalInput")
    o_d = nc.dram_tensor(
        "out", [128, IMG_PER_CORE * QOUT], mybir.dt.bfloat16, kind="ExternalOutput"
    )

    ident = mybir.ActivationFunctionType.Identity

    with tile.TileContext(nc) as tc:
        with (
            tc.tile_pool(name="const", bufs=1) as cpool,
            tc.tile_pool(name="xin", bufs=3) as xpool,
            tc.tile_pool(name="stage", bufs=2) as spool,
            tc.tile_pool(name="psum", bufs=3, space="PSUM") as ppool,
        ):
            # w/bias ride the SWDGE ring so they overlap the first x chunk
            # on the HWDGE ring.
            w_sb = cpool.tile([128, NTAPS * 128], mybir.dt.bfloat16, tag="w")
            nc.gpsimd.dma_start(w_sb[:], w_d[:])
            b_sb = cpool.tile([128, 1], mybir.dt.float32, tag="b")
            nc.gpsimd.dma_start(b_sb[:], b_d[:])

            # PE warm-up: ~5us of dummy matmuls on a zeroed tile so the HAM
            # clock-gate releases before the real stream begins (results are
            # never read).
            warm_src = cpool.tile([64, SLOT], mybir.dt.bfloat16, tag="warm")
            nc.vector.memset(warm_src[:], 0.0)
            warm_ps = ppool.tile([128, SLOT], mybir.dt.float32, tag="pswarm", bufs=1)
            N_WARM = 24
            for i in range(N_WARM):
                nc.tensor.matmul(
                    warm_ps[:, :],
                    lhsT=warm_src[:, 0:128],
                    rhs=warm_src[:, :],
                    start=(i == 0),
                    stop=(i == N_WARM - 1),
                )

            chunk_starts = set(_chunk_starts())
            xt = None
            st_a = st_b = None
            for s in range(N_SLOTS):
                q0 = s * SLOT
                n = min(SLOT, QOUT - q0)

                if s in chunk_starts:
                    cq0 = q0
                    nslots = FIRST_CHUNK_SLOTS if s == 0 else SLOTS_PER_CHUNK
                    ext = min(QOUT, cq0 + nslots * SLOT) - cq0 + HALO
                    xt = xpool.tile([128, XTILE_COLS], mybir.dt.bfloat16, tag="x")
                    nc.sync.dma_start(xt[:, :ext], x_d[:, cq0 : cq0 + ext])

                if s % STAGE_SLOTS == 0:
                    g0 = q0
                    gext = min(QOUT, g0 + STAGE_Q) - g0
                    st_a = spool.tile([128, STAGE_Q], mybir.dt.bfloat16, tag="sa")
                    st_b = spool.tile([128, STAGE_Q], mybir.dt.bfloat16, tag="sb")

                psa = ppool.tile([128, SLOT], mybir.dt.float32, tag="psa")
                psb = ppool.tile([128, SLOT], mybir.dt.float32, tag="psb")

                for t in range(NTAPS):
                    dh, dw = divmod(t, 3)
                    lo = q0 - cq0 + dh * PC + dw
                    first, last = t == 0, t == NTAPS - 1
                    nc.tensor.matmul(
                        psa[:, :n],
                        lhsT=w_sb[0:64, t * 128 : (t + 1) * 128],
                        rhs=xt[0:64, lo : lo + n],
                        start=first,
                        stop=last,
                    )
                    nc.tensor.matmul(
                        psb[:, :n],
                        lhsT=w_sb[64:128, t * 128 : (t + 1) * 128],
                        rhs=xt[64:128, lo : lo + n],
                        start=first,
                        stop=last,
                    )

                so = q0 - g0
                nc.vector.tensor_scalar_add(st_a[:, so : so + n], psa[:, :n], b_sb[:])
                nc.scalar.activation(st_b[:, so : so + n], psb[:, :n], ident, bias=b_sb[:])

                if s % STAGE_SLOTS == STAGE_SLOTS - 1 or s == N_SLOTS - 1:
                    nc.sync.dma_start(o_d[:, g0 : g0 + gext], st_a[:, :gext])
                    nc.sync.dma_start(
                        o_d[:, QOUT + g0 : QOUT + g0 + gext], st_b[:, :gext]
                    )

    nc.compile()
    return nc


def _get_nc():
    global _COMPILED
    if _COMPILED is None:
        _COMPILED = _build()
    return _COMPILED


def kernel(x: np.ndarray, W: np.ndarray, b: np.ndarray) -> np.ndarray:
    from concourse.bass_utils import run_bass_kernel_spmd

    nc = _get_nc()

    xb = np.asarray(x, dtype=np.float32).astype(BF16)
    X = np.zeros((N_CORES, IMG_PER_CORE, C_IN, PR, PC), BF16)
    X[:, :, :, 1 : H + 1, 1 : W_IMG + 1] = xb.reshape(
        N_CORES, IMG_PER_CORE, H, W_IMG, C_IN
    ).transpose(0, 1, 4, 2, 3)
    Xf = X.reshape(N_CORES, 128, PLANE)

    Wb = np.sign(np.asarray(W, dtype=np.float32)).astype(BF16).reshape(NTAPS, C_IN, C_OUT)
    wh = np.empty((2, C_IN, NTAPS, C_OUT), BF16)
    wh[:] = Wb.transpose(1, 0, 2)[None]
    wh = np.ascontiguousarray(wh.reshape(128, NTAPS * C_OUT))

    bh = np.ascontiguousarray(np.asarray(b, dtype=np.float32).reshape(128, 1))

    in_maps = [{"x": Xf[c], "w": wh, "b": bh} for c in range(N_CORES)]
    res = run_bass_kernel_spmd(nc, in_maps, list(range(N_CORES)))
    global _LAST_RES
    _LAST_RES = res

    O = np.stack([res.results[c]["out"] for c in range(N_CORES)])
    O = O.reshape(N_CORES, C_OUT, IMG_PER_CORE, H, PC)[:, :, :, :, :W_IMG]
    y = O.transpose(0, 2, 3, 4, 1).reshape(16, H, W_IMG, C_OUT)
    return np.ascontiguousarray(y).astype(np.float32)


# revision 13
# speedup vs baseline: 1.0097x; 1.0097x over previous
"""BinaryConv2d (3x3, SAME, NHWC) on 8 trn2 NeuronCores.

Sharding: data-parallel over batch — 2 images per core; the tiny binarized
weight tensor is replicated. Per core, the two images are packed on the two
64-partition halves of SBUF so each 3x3-tap matmul pair (K=64 contraction =
C_in) runs concurrently on disjoint row-groups of the 128x128 PE array.

Layout trick: the conv is evaluated on a flat q-grid over a zero-padded
226-wide plane, so each of the 9 taps is a pure free-dim offset
(dh*226 + dw) into the same SBUF x tile; row-crossing outputs land in 2
garbage columns per row that the host discards.
"""

import sys

for _p in ("/opt/trn_rl_repo",):
    if _p not in sys.path:
        sys.path.insert(0, _p)

import ml_dtypes
import numpy as np

BF16 = ml_dtypes.bfloat16

N_CORES = 8
IMG_PER_CORE = 2
H = W_IMG = 224
C_IN, C_OUT = 64, 128
PR, PC = 227, 226  # padded plane: 226 rows of data + 1 extra zero row
PLANE = PR * PC  # 51302
QOUT = H * PC  # 50624 q-positions per image (2 garbage cols per row)
NTAPS = 9
SLOT = 512  # q-positions per matmul (one PSUM bank of fp32)
N_SLOTS = (QOUT + SLOT - 1) // SLOT  # 99 (last slot = 448)
SLOTS_PER_CHUNK = 16
FIRST_CHUNK_SLOTS = 2  # small first chunk so the PE starts early
HALO = 2 * PC + 2  # 454: max tap offset
CHUNK_Q = SLOTS_PER_CHUNK * SLOT
XTILE_COLS = CHUNK_Q + HALO
STAGE_SLOTS = 8
STAGE_Q = STAGE_SLOTS * SLOT


def _chunk_starts():
    starts = [0, FIRST_CHUNK_SLOTS]
    while starts[-1] + SLOTS_PER_CHUNK < N_SLOTS:
        starts.append(starts[-1] + SLOTS_PER_CHUNK)
    return starts

_COMPILED = None
_LAST_RES = None


def _build():
    import concourse.mybir as mybir
    import concourse.tile as tile
    from concourse import bacc

    nc = bacc.Bacc(
        "TRN2", target_bir_lowering=False, debug=False, num_devices=N_CORES
    )
    x_d = nc.dram_tensor("x", [128, PLANE], mybir.dt.bfloat16, kind="ExternalInput")
    w_d = nc.dram_tensor(
        "w", [128, NTAPS * 128], mybir.dt.bfloat16, kind="ExternalInput"
    )
    b_d = nc.dram_tensor("b", [128, 1], mybir.dt.float32, kind="ExternalInput")
    o_d = nc.dram_tensor(
        "out", [128, IMG_PER_CORE * QOUT], mybir.dt.bfloat16, kind="ExternalOutput"
    )

    ident = mybir.ActivationFunctionType.Identity

    with tile.TileContext(nc) as tc:
        with (
            tc.tile_pool(name="const", bufs=1) as cpool,
            tc.tile_pool(name="xin", bufs=3) as xpool,
            tc.tile_pool(name="stage", bufs=2) as spool,
            tc.tile_pool(name="psum", bufs=4, space="PSUM") as ppool,
        ):
            # w/bias ride the SWDGE ring so they overlap the first x chunk
            # on the HWDGE ring.
            w_sb = cpool.tile([128, NTAPS * 128], mybir.dt.bfloat16, tag="w")
            nc.gpsimd.dma_start(w_sb[:], w_d[:])
            b_sb = cpool.tile([128, 1], mybir.dt.float32, tag="b")
            nc.gpsimd.dma_start(b_sb[:], b_d[:])

            chunk_starts = set(_chunk_starts())
            xt = None
            st_a = st_b = None
            for s in range(N_SLOTS):
                q0 = s * SLOT
                n = min(SLOT, QOUT - q0)

                if s in chunk_starts:
                    cq0 = q0
                    nslots = FIRST_CHUNK_SLOTS if s == 0 else SLOTS_PER_CHUNK
                    ext = min(QOUT, cq0 + nslots * SLOT) - cq0 + HALO
                    xt = xpool.tile([128, XTILE_COLS], mybir.dt.bfloat16, tag="x")
                    nc.sync.dma_start(xt[:, :ext], x_d[:, cq0 : cq0 + ext])

                if s % STAGE_SLOTS == 0:
                    g0 = q0
                    gext = min(QOUT, g0 + STAGE_Q) - g0
                    st_a = spool.tile([128, STAGE_Q], mybir.dt.bfloat16, tag="sa")
                    st_b = spool.tile([128, STAGE_Q], mybir.dt.bfloat16, tag="sb")

                psa = ppool.tile([128, SLOT], mybir.dt.float32, tag="psa")
                psb = ppool.tile([128, SLOT], mybir.dt.float32, tag="psb")

                for t in range(NTAPS):
                    dh, dw = divmod(t, 3)
                    lo = q0 - cq0 + dh * PC + dw
                    first, last = t == 0, t == NTAPS - 1
                    nc.tensor.matmul(
                        psa[:, :n],
                        lhsT=w_sb[0:64, t * 128 : (t + 1) * 128],
                        rhs=xt[0:64, lo : lo + n],
                        start=first,
                        stop=last,
                    )
                    nc.tensor.matmul(
                        psb[:, :n],
                        lhsT=w_sb[64:128, t * 128 : (t + 1) * 128],
                        rhs=xt[64:128, lo : lo + n],
                        start=first,
                        stop=last,
                    )

                so = q0 - g0
                nc.vector.tensor_scalar_add(st_a[:, so : so + n], psa[:, :n], b_sb[:])
                nc.scalar.activation(st_b[:, so : so + n], psb[:, :n], ident, bias=b_sb[:])

                if s % STAGE_SLOTS == STAGE_SLOTS - 1 or s == N_SLOTS - 1:
                    nc.sync.dma_start(o_d[:, g0 : g0 + gext], st_a[:, :gext])
                    nc.sync.dma_start(
                        o_d[:, QOUT + g0 : QOUT + g0 + gext], st_b[:, :gext]
                    )

    nc.compile()
    return nc


def _get_nc():
    global _COMPILED
    if _COMPILED is None:
        _COMPILED = _build()
    return _COMPILED


def kernel(x: np.ndarray, W: np.ndarray, b: np.ndarray) -> np.ndarray:
    from concourse.bass_utils import run_bass_kernel_spmd

    nc = _get_nc()

    xb = np.asarray(x, dtype=np.float32).astype(BF16)
    X = np.zeros((N_CORES, IMG_PER_CORE, C_IN, PR, PC), BF16)
    X[:, :, :, 1 : H + 1, 1 : W_IMG + 1] = xb.reshape(
        N_CORES, IMG_PER_CORE, H, W_IMG, C_IN
    ).transpose(0, 1, 4, 2, 3)
    Xf = X.reshape(N_CORES, 128, PLANE)

    Wb = np.sign(np.asarray(W, dtype=np.float32)).astype(BF16).reshape(NTAPS, C_IN, C_OUT)
    wh = np.empty((2, C_IN, NTAPS, C_OUT), BF16)
    wh[:] = Wb.transpose(1, 0, 2)[None]
    wh = np.ascontiguousarray(wh.reshape(128, NTAPS * C_OUT))

    bh = np.ascontiguousarray(np.asarray(b, dtype=np.float32).reshape(128, 1))

    in_maps = [{"x": Xf[c], "w": wh, "b": bh} for c in range(N_CORES)]
    res = run_bass_kernel_spmd(nc, in_maps, list(range(N_CORES)))
    global _LAST_RES
    _LAST_RES = res

    O = np.stack([res.results[c]["out"] for c in range(N_CORES)])
    O = O.reshape(N_CORES, C_OUT, IMG_PER_CORE, H, PC)[:, :, :, :, :W_IMG]
    y = O.transpose(0, 2, 3, 4, 1).reshape(16, H, W_IMG, C_OUT)
    return np.ascontiguousarray(y).astype(np.float32)


# revision 17
# speedup vs baseline: 1.0133x; 1.0036x over previous
"""BinaryConv2d (3x3, SAME, NHWC) on 8 trn2 NeuronCores.

Sharding: data-parallel over batch — 2 images per core; the tiny binarized
weight tensor is replicated. Per core, the two images are packed on the two
64-partition halves of SBUF so each 3x3-tap matmul pair (K=64 contraction =
C_in) runs concurrently on disjoint row-groups of the 128x128 PE array.

Layout trick: the conv is evaluated on a flat q-grid over a zero-padded
226-wide plane, so each of the 9 taps is a pure free-dim offset
(dh*226 + dw) into the same SBUF x tile; row-crossing outputs land in 2
garbage columns per row that the host discards.
"""

import sys

for _p in ("/opt/trn_rl_repo",):
    if _p not in sys.path:
        sys.path.insert(0, _p)

import ml_dtypes
import numpy as np

BF16 = ml_dtypes.bfloat16

N_CORES = 8
IMG_PER_CORE = 2
H = W_IMG = 224
C_IN, C_OUT = 64, 128
PR, PC = 227, 226  # padded plane: 226 rows of data + 1 extra zero row
PLANE = PR * PC  # 51302
QOUT = H * PC  # 50624 q-positions per image (2 garbage cols per row)
NTAPS = 9
SLOT = 512  # q-positions per matmul (one PSUM bank of fp32)
N_SLOTS = (QOUT + SLOT - 1) // SLOT  # 99 (last slot = 448)
SLOTS_PER_CHUNK = 16
HALO = 2 * PC + 2  # 454: max tap offset
CHUNK_Q = SLOTS_PER_CHUNK * SLOT
XTILE_COLS = CHUNK_Q + HALO
STAGE_SLOTS = 8
STAGE_Q = STAGE_SLOTS * SLOT


def _chunk_starts():
    starts = [0, 1, 2]
    while starts[-1] + SLOTS_PER_CHUNK < N_SLOTS:
        starts.append(starts[-1] + SLOTS_PER_CHUNK)
    return starts

_COMPILED = None
_LAST_RES = None


def _build():
    import concourse.mybir as mybir
    import concourse.tile as tile
    from concourse import bacc

    nc = bacc.Bacc(
        "TRN2", target_bir_lowering=False, debug=False, num_devices=N_CORES
    )
    x_d = nc.dram_tensor("x", [128, PLANE], mybir.dt.bfloat16, kind="ExternalInput")
    w_d = nc.dram_tensor(
        "w", [128, NTAPS * 128], mybir.dt.bfloat16, kind="ExternalInput"
    )
    b_d = nc.dram_tensor("b", [128, 1], mybir.dt.float32, kind="ExternalInput")
    o_d = nc.dram_tensor(
        "out", [128, IMG_PER_CORE * QOUT], mybir.dt.bfloat16, kind="ExternalOutput"
    )

    ident = mybir.ActivationFunctionType.Identity

    with tile.TileContext(nc) as tc:
        with (
            tc.tile_pool(name="const", bufs=1) as cpool,
            tc.tile_pool(name="xin", bufs=3) as xpool,
            tc.tile_pool(name="stage", bufs=2) as spool,
            tc.tile_pool(name="psum", bufs=4, space="PSUM") as ppool,
        ):
            # Critical-path-first ordering on the HWDGE ring: tap-0 weights,
            # then (inside the loop) the 1-slot first x chunk, then the rest
            # of the weights + bias, then the remaining chunks.
            w_sb = cpool.tile([128, NTAPS * 128], mybir.dt.bfloat16, tag="w")
            nc.sync.dma_start(w_sb[:, :128], w_d[:, :128])
            b_sb = cpool.tile([128, 1], mybir.dt.float32, tag="b")

            chunk_starts = set(_chunk_starts())
            xt = None
            st_a = st_b = None
            for s in range(N_SLOTS):
                q0 = s * SLOT
                n = min(SLOT, QOUT - q0)

                if s in chunk_starts:
                    cq0 = q0
                    nslots = 1 if s < 2 else SLOTS_PER_CHUNK
                    ext = min(QOUT, cq0 + nslots * SLOT) - cq0 + HALO
                    xt = xpool.tile([128, XTILE_COLS], mybir.dt.bfloat16, tag="x")
                    nc.sync.dma_start(xt[:, :ext], x_d[:, cq0 : cq0 + ext])
                    if s == 0:
                        nc.sync.dma_start(w_sb[:, 128:], w_d[:, 128:])
                        nc.sync.dma_start(b_sb[:], b_d[:])

                if s % STAGE_SLOTS == 0:
                    g0 = q0
                    gext = min(QOUT, g0 + STAGE_Q) - g0
                    st_a = spool.tile([128, STAGE_Q], mybir.dt.bfloat16, tag="sa")
                    st_b = spool.tile([128, STAGE_Q], mybir.dt.bfloat16, tag="sb")

                psa = ppool.tile([128, SLOT], mybir.dt.float32, tag="psa")
                psb = ppool.tile([128, SLOT], mybir.dt.float32, tag="psb")

                for t in range(NTAPS):
                    dh, dw = divmod(t, 3)
                    lo = q0 - cq0 + dh * PC + dw
                    first, last = t == 0, t == NTAPS - 1
                    nc.tensor.matmul(
                        psa[:, :n],
                        lhsT=w_sb[0:64, t * 128 : (t + 1) * 128],
                        rhs=xt[0:64, lo : lo + n],
                        start=first,
                        stop=last,
                    )
                    nc.tensor.matmul(
                        psb[:, :n],
                        lhsT=w_sb[64:128, t * 128 : (t + 1) * 128],
                        rhs=xt[64:128, lo : lo + n],
                        start=first,
                        stop=last,
                    )

                so = q0 - g0
                nc.vector.tensor_scalar_add(st_a[:, so : so + n], psa[:, :n], b_sb[:])
                nc.scalar.activation(st_b[:, so : so + n], psb[:, :n], ident, bias=b_sb[:])

                if s % STAGE_SLOTS == STAGE_SLOTS - 1 or s == N_SLOTS - 1:
                    nc.sync.dma_start(o_d[:, g0 : g0 + gext], st_a[:, :gext])
                    nc.sync.dma_start(
                        o_d[:, QOUT + g0 : QOUT + g0 + gext], st_b[:, :gext]
                    )

    nc.compile()
    return nc


def _get_nc():
    global _COMPILED
    if _COMPILED is None:
        _COMPILED = _build()
    return _COMPILED


def kernel(x: np.ndarray, W: np.ndarray, b: np.ndarray) -> np.ndarray:
    from concourse.bass_utils import run_bass_kernel_spmd

    nc = _get_nc()

    xb = np.asarray(x, dtype=np.float32).astype(BF16)
    X = np.zeros((N_CORES, IMG_PER_CORE, C_IN, PR, PC), BF16)
    X[:, :, :, 1 : H + 1, 1 : W_IMG + 1] = xb.reshape(
        N_CORES, IMG_PER_CORE, H, W_IMG, C_IN
    ).transpose(0, 1, 4, 2, 3)
    Xf = X.reshape(N_CORES, 128, PLANE)

    Wb = np.sign(np.asarray(W, dtype=np.float32)).astype(BF16).reshape(NTAPS, C_IN, C_OUT)
    wh = np.empty((2, C_IN, NTAPS, C_OUT), BF16)
    wh[:] = Wb.transpose(1, 0, 2)[None]
    wh = np.ascontiguousarray(wh.reshape(128, NTAPS * C_OUT))

    bh = np.ascontiguousarray(np.asarray(b, dtype=np.float32).reshape(128, 1))

    in_maps = [{"x": Xf[c], "w": wh, "b": bh} for c in range(N_CORES)]
    res = run_bass_kernel_spmd(nc, in_maps, list(range(N_CORES)))
    global _LAST_RES
    _LAST_RES = res

    O = np.stack([res.results[c]["out"] for c in range(N_CORES)])
    O = O.reshape(N_CORES, C_OUT, IMG_PER_CORE, H, PC)[:, :, :, :, :W_IMG]
    y = O.transpose(0, 2, 3, 4, 1).reshape(16, H, W_IMG, C_OUT)
    return np.ascontiguousarray(y).astype(np.float32)


# revision 21
# speedup vs baseline: 1.0250x; 1.0115x over previous
"""BinaryConv2d (3x3, SAME, NHWC) on 8 trn2 NeuronCores.

Sharding: data-parallel over batch — 2 images per core; the tiny binarized
weight tensor is replicated. Per core, the two images are packed on the two
64-partition halves of SBUF so each 3x3-tap matmul pair (K=64 contraction =
C_in) runs concurrently on disjoint row-groups of the 128x128 PE array.

Layout trick: the conv is evaluated on a flat q-grid over a zero-padded
226-wide plane, so each of the 9 taps is a pure free-dim offset
(dh*226 + dw) into the same SBUF x tile; row-crossing outputs land in 2
garbage columns per row that the host discards.
"""

import sys

for _p in ("/opt/trn_rl_repo",):
    if _p not in sys.path:
        sys.path.insert(0, _p)

import ml_dtypes
import numpy as np

BF16 = ml_dtypes.bfloat16

N_CORES = 8
IMG_PER_CORE = 2
H = W_IMG = 224
C_IN, C_OUT = 64, 128
PR, PC = 227, 226  # padded plane: 226 rows of data + 1 extra zero row
PLANE = PR * PC  # 51302
QOUT = H * PC  # 50624 q-positions per image (2 garbage cols per row)
NTAPS = 9
SLOT = 512  # q-positions per matmul (one PSUM bank of fp32)
N_SLOTS = (QOUT + SLOT - 1) // SLOT  # 99 (last slot = 448)
SLOTS_PER_CHUNK = 16
HALO = 2 * PC + 2  # 454: max tap offset
CHUNK_Q = SLOTS_PER_CHUNK * SLOT
XTILE_COLS = CHUNK_Q + HALO
STAGE_SLOTS = 8
STAGE_Q = STAGE_SLOTS * SLOT


def _chunk_plan():
    """(start_slot -> n_slots): geometric ramp so early chunks land
    just-in-time, then steady 16-slot chunks."""
    plan = {}
    s, size = 0, 1
    while s < N_SLOTS:
        n = min(size, N_SLOTS - s, SLOTS_PER_CHUNK)
        plan[s] = n
        s += n
        if size < SLOTS_PER_CHUNK:
            size = size * 2 if s >= 2 else 1
    return plan

_COMPILED = None
_LAST_RES = None


def _build():
    import concourse.mybir as mybir
    import concourse.tile as tile
    from concourse import bacc

    nc = bacc.Bacc(
        "TRN2", target_bir_lowering=False, debug=False, num_devices=N_CORES
    )
    x_d = nc.dram_tensor("x", [128, PLANE], mybir.dt.bfloat16, kind="ExternalInput")
    w_d = nc.dram_tensor(
        "w", [128, NTAPS * 128], mybir.dt.bfloat16, kind="ExternalInput"
    )
    b_d = nc.dram_tensor("b", [128, 1], mybir.dt.float32, kind="ExternalInput")
    o_d = nc.dram_tensor(
        "out", [128, IMG_PER_CORE * QOUT], mybir.dt.bfloat16, kind="ExternalOutput"
    )

    ident = mybir.ActivationFunctionType.Identity

    with tile.TileContext(nc) as tc:
        with (
            tc.tile_pool(name="const", bufs=1) as cpool,
            tc.tile_pool(name="xin", bufs=4) as xpool,
            tc.tile_pool(name="stage", bufs=2) as spool,
            tc.tile_pool(name="psum", bufs=4, space="PSUM") as ppool,
        ):
            # Critical-path-first ordering on the HWDGE ring: tap-0 weights,
            # then (inside the loop) the 1-slot first x chunk, then the rest
            # of the weights + bias, then the remaining chunks.
            w_sb = cpool.tile([128, NTAPS * 128], mybir.dt.bfloat16, tag="w")
            nc.sync.dma_start(w_sb[:, :128], w_d[:, :128])
            b_sb = cpool.tile([128, 1], mybir.dt.float32, tag="b")

            chunk_plan = _chunk_plan()
            xt = None
            st_a = st_b = None
            for s in range(N_SLOTS):
                q0 = s * SLOT
                n = min(SLOT, QOUT - q0)

                if s in chunk_plan:
                    cq0 = q0
                    ext = min(QOUT, cq0 + chunk_plan[s] * SLOT) - cq0 + HALO
                    xt = xpool.tile([128, XTILE_COLS], mybir.dt.bfloat16, tag="x")
                    nc.sync.dma_start(xt[:, :ext], x_d[:, cq0 : cq0 + ext])
                    if s == 0:
                        # remaining weights right behind the first x slot
                        # (must be emitted before slot 0's matmuls read them)
                        nc.sync.dma_start(w_sb[:, 128:384], w_d[:, 128:384])
                        nc.sync.dma_start(w_sb[:, 384:], w_d[:, 384:])
                        nc.sync.dma_start(b_sb[:], b_d[:])

                if s % STAGE_SLOTS == 0:
                    g0 = q0
                    gext = min(QOUT, g0 + STAGE_Q) - g0
                    st_a = spool.tile([128, STAGE_Q], mybir.dt.bfloat16, tag="sa")
                    st_b = spool.tile([128, STAGE_Q], mybir.dt.bfloat16, tag="sb")

                psa = ppool.tile([128, SLOT], mybir.dt.float32, tag="psa")
                psb = ppool.tile([128, SLOT], mybir.dt.float32, tag="psb")

                for t in range(NTAPS):
                    dh, dw = divmod(t, 3)
                    lo = q0 - cq0 + dh * PC + dw
                    first, last = t == 0, t == NTAPS - 1
                    nc.tensor.matmul(
                        psa[:, :n],
                        lhsT=w_sb[0:64, t * 128 : (t + 1) * 128],
                        rhs=xt[0:64, lo : lo + n],
                        start=first,
                        stop=last,
                    )
                    nc.tensor.matmul(
                        psb[:, :n],
                        lhsT=w_sb[64:128, t * 128 : (t + 1) * 128],
                        rhs=xt[64:128, lo : lo + n],
                        start=first,
                        stop=last,
                    )

                so = q0 - g0
                nc.vector.tensor_scalar_add(st_a[:, so : so + n], psa[:, :n], b_sb[:])
                nc.scalar.activation(st_b[:, so : so + n], psb[:, :n], ident, bias=b_sb[:])

                if s % STAGE_SLOTS == STAGE_SLOTS - 1 or s == N_SLOTS - 1:
                    nc.sync.dma_start(o_d[:, g0 : g0 + gext], st_a[:, :gext])
                    nc.sync.dma_start(
                        o_d[:, QOUT + g0 : QOUT + g0 + gext], st_b[:, :gext]
                    )

    nc.compile()
    return nc


def _get_nc():
    global _COMPILED
    if _COMPILED is None:
        _COMPILED = _build()
    return _COMPILED


def kernel(x: np.ndarray, W: np.ndarray, b: np.ndarray) -> np.ndarray:
    from concourse.bass_utils import run_bass_kernel_spmd

    nc = _get_nc()

    xb = np.asarray(x, dtype=np.float32).astype(BF16)
    X = np.zeros((N_CORES, IMG_PER_CORE, C_IN, PR, PC), BF16)
    X[:, :, :, 1 : H + 1, 1 : W_IMG + 1] = xb.reshape(
        N_CORES, IMG_PER_CORE, H, W_IMG, C_IN
    ).transpose(0, 1, 4, 2, 3)
    Xf = X.reshape(N_CORES, 128, PLANE)

    Wb = np.sign(np.asarray(W, dtype=np.float32)).astype(BF16).reshape(NTAPS, C_IN, C_OUT)
    wh = np.empty((2, C_IN, NTAPS, C_OUT), BF16)
    wh[:] = Wb.transpose(1, 0, 2)[None]
    wh = np.ascontiguousarray(wh.reshape(128, NTAPS * C_OUT))

    bh = np.ascontiguousarray(np.asarray(b, dtype=np.float32).reshape(128, 1))

    in_maps = [{"x": Xf[c], "w": wh, "b": bh} for c in range(N_CORES)]
    res = run_bass_kernel_spmd(nc, in_maps, list(range(N_CORES)))
    global _LAST_RES
    _LAST_RES = res

    O = np.stack([res.results[c]["out"] for c in range(N_CORES)])
    O = O.reshape(N_CORES, C_OUT, IMG_PER_CORE, H, PC)[:, :, :, :, :W_IMG]
    y = O.transpose(0, 2, 3, 4, 1).reshape(16, H, W_IMG, C_OUT)
    return np.ascontiguousarray(y).astype(np.float32)
